# revision 26
# baseline (speedup 1.0000x reference)
"""Trainium2 Bass kernel for nn_AttentionKernelIntegral (linear attention
with instance-normed k/v, collapsed algebraically).

Math
----
Reference computes (per batch, H=8 heads, D=64, C=OUT=256, N=16384):
    q = u @ Wq^T ; k = u @ Wk^T ; v = u @ Wv^T          (per head blocks)
    khat = instnorm_n(k); vhat = instnorm_n(v)
    kv_h = (1/N) khat_h^T vhat_h                        [D, D]
    out  = concat_h(q_h @ kv_h) @ Wo^T + bo

Everything downstream of u is linear except the instance-norm statistics
(exact functions of first/second moments over n), so the network
collapses to   out = u @ W_eff + bo.  With the *centered* covariance

    Ctilde = (Cuu - su su^T / N) / N,   Cuu = u^T u, su = u^T 1

the means drop out entirely:

    kv_h   = Dk_h (Wk_h Ctilde Wv_h^T) Dv_h
    vark_d = (Wk Ctilde Wk^T)_dd ;  Dk = diag(rsqrt(vark + eps))
    W_eff  = sum_h Wq_h^T kv_h Wo_h^T                   [C, OUT]

Sharding: 8 cores = 4 batches x 2 grid-halves.  Each core receives the
full u for its batch (bf16, with ITS half permuted first), accumulates
Cuu over the full grid, and emits out^T for its own half.

Layouts: the host pre-packs u / weights / output DRAM tensors
partition-major so every DMA descriptor moves 2-8 KB contiguous per
partition.  u and weights are bf16 (host cast); output is stored bf16
(out^T) and upcast + unpermuted on the host.

Cuu uses symmetry: the row-block-1 matmul streams only cols 128..256;
the missing [128,128] block of Ctilde is reconstructed by one PE
transpose.  The -su su^T/N correction is accumulated straight onto the
Cuu PSUM banks by two K=1 matmuls.  Variances are produced directly in
column format (N=1 matmuls against a ones column); rv is folded into
the per-pair block-diag mask and rk into the bx copy, so no scaled
weight copies are needed.
"""

import numpy as np
import ml_dtypes

import concourse.bass as bass
import concourse.tile as tile
from concourse import bacc, mybir
from concourse.bass_utils import run_bass_kernel_spmd
from concourse.masks import make_identity, make_block_diagonal

F32 = mybir.dt.float32
BF16 = mybir.dt.bfloat16
AL = mybir.AluOpType
AF = mybir.ActivationFunctionType

P = 128
N_FULL = 16384
N_HALF = 8192
C = 256
HD = 512          # H * D
OUT = 256
EPS = 1e-5
CH_ROWS = 2048
N_CHUNKS = N_FULL // CH_ROWS      # 8 chunks of 2048 rows (full grid)
SUBT = CH_ROWS // P               # 16 row-subtiles per chunk
G_ALL = N_FULL // P               # 128 row-tiles total
G_MINE = N_HALF // P              # first 64 belong to this core
INV_N = 1.0 / float(N_FULL)
GROUP = 512                       # phase-3 column group of out^T
NGROUPS = N_HALF // GROUP         # 16
OCH = 4                           # phase-3 store chunks (4 groups each)
C1 = C + 1                        # u row + embedded 1.0 (ones column)
U_ROW = N_CHUNKS * SUBT * C1      # per-partition elements of u_r
O_ROW = OCH * 2 * CH_ROWS         # 16384 per-partition elements of out_r


def build_nc():
    nc = bacc.Bacc(
        "TRN2",
        target_bir_lowering=False,
        debug=False,
        num_devices=8,
    )
    u_d = nc.dram_tensor("u", [P, U_ROW], BF16, kind="ExternalInput").ap()
    wq_d = nc.dram_tensor("wq", [P, 4 * C], BF16, kind="ExternalInput").ap()
    wkv_d = nc.dram_tensor("wkv", [P, 2 * 2 * HD], BF16, kind="ExternalInput").ap()
    wot_d = nc.dram_tensor("wot", [P, 4 * OUT], BF16, kind="ExternalInput").ap()
    out_d = nc.dram_tensor("out", [P, O_ROW], BF16, kind="ExternalOutput").ap()

    with tile.TileContext(nc) as tc:
        with tc.tile_pool(name="pers", bufs=1) as pers:
            # ---- persistent tiles -------------------------------------
            uT = pers.tile([P, 2, N_HALF], BF16)         # u^T (bf16, own half)
            ident = pers.tile([P, P], F32)
            make_identity(nc, ident[:])
            ident_bf = pers.tile([P, P], BF16)
            nc.vector.tensor_copy(ident_bf[:], ident[:])
            mask_f = pers.tile([P, P], F32)
            make_block_diagonal(nc, mask_f[:], 64)
            mask_bf = pers.tile([P, P], BF16)
            nc.vector.tensor_copy(mask_bf[:], mask_f[:])
            wq_bf = pers.tile([P, 4, C], BF16)           # Wq natural [hd, c]
            wkvT_bf = pers.tile([P, 2, 2 * HD], BF16)    # [Wk^T | Wv^T] [c, 2hd]
            woT_bf = pers.tile([P, 4, OUT], BF16)        # Wo^T  [hd, o]
            weff = pers.tile([P, 2, OUT], BF16)
            ct_bf = pers.tile([P, 2, C], BF16)           # Ctilde (bf16)
            ones_bf = pers.tile([P, 1], BF16)
            nc.vector.memset(ones_bf[:], 1.0)
            eps_col = pers.tile([P, 1], F32)
            nc.vector.memset(eps_col[:], EPS)
            su_col = pers.tile([P, 2], BF16)             # su / N
            su_nrow = pers.tile([1, C], BF16)            # -su / N
            rk_col = pers.tile([P, 4], F32)
            rv_col = pers.tile([P, 4], F32)
            mask_rv = pers.tile([P, 4, P], BF16)         # mask * rv (per pair)
            # prewarm ACT tables used later (Copy via scalar.mul, Sqrt)
            warm = pers.tile([1, 8], F32)
            nc.vector.memset(warm[:], 1.0)
            nc.scalar.mul(warm[:], warm[:], 1.0)
            nc.scalar.activation(warm[:], warm[:], AF.Sqrt)

            # ---- phase 1: stream u, accumulate Cuu, transpose own half
            with (
                tc.tile_pool(name="upool", bufs=3) as upool,
                tc.tile_pool(name="pacc", bufs=1, space="PSUM") as pacc,
                tc.tile_pool(name="ptr", bufs=3, space="PSUM") as ptr,
            ):
                cps0 = pacc.tile([P, C + 1], F32, tag="c0", name="c0")
                cps1 = pacc.tile([P, C + 1 - P], F32, tag="c1", name="c1")
                # chunk 0 arrives in three j-slices so the PE starts early
                sched = [(0, 0, 4), (0, 4, 4), (0, 8, 8)]
                for ch in range(1, N_CHUNKS):
                    sched.append((ch, 0, SUBT))
                ub = None
                first_dmas = 0
                for ch, j0, nsub in sched:
                    if j0 == 0:
                        ub = upool.tile([P, SUBT, C1], BF16, tag="ub", name="ub")
                    src_ap = u_d[:, ch * SUBT * C1:(ch + 1) * SUBT * C1].rearrange(
                        "p (j c) -> p j c", c=C1
                    )
                    nc.sync.dma_start(
                        ub[:, j0:j0 + nsub, :], src_ap[:, j0:j0 + nsub, :]
                    )
                    if (ch, j0) == (N_CHUNKS - 1, 0):
                        # weights are phase-2 inputs; issue their DMAs
                        # after the u stream so they don't delay it
                        nc.sync.dma_start(
                            wq_bf[:], wq_d.rearrange("p (a c) -> p a c", c=C)
                        )
                        nc.sync.dma_start(
                            wkvT_bf[:],
                            wkv_d.rearrange("p (a c) -> p a c", c=2 * HD),
                        )
                        nc.sync.dma_start(
                            woT_bf[:], wot_d.rearrange("p (a c) -> p a c", c=OUT)
                        )
                    for j in range(j0, j0 + nsub):
                        g = ch * SUBT + j
                        nc.tensor.matmul(
                            cps0[:],
                            ub[:, j, 0:P],
                            ub[:, j, :],
                            start=(g == 0),
                            stop=(g == G_ALL - 1),
                        )
                        nc.tensor.matmul(
                            cps1[:],
                            ub[:, j, P:C],
                            ub[:, j, P:C + 1],
                            start=(g == 0),
                            stop=(g == G_ALL - 1),
                        )
                        if g < G_MINE:
                            tps = ptr.tile([P, C], BF16, tag="uT", name="tps")
                            for t in range(2):
                                nc.tensor.transpose(
                                    tps[:, t * P:(t + 1) * P],
                                    ub[:, j, t * P:(t + 1) * P],
                                    ident_bf[:],
                                )
                            if g % 2 == 0:
                                nc.vector.tensor_copy(
                                    uT[:, :, g * P:(g + 1) * P],
                                    tps[:].rearrange("p (t n) -> p t n", t=2),
                                )
                            else:
                                nc.scalar.copy(
                                    uT[:, :, g * P:(g + 1) * P],
                                    tps[:].rearrange("p (t n) -> p t n", t=2),
                                )

                # ---- C0 = Cuu/N (bf16) + su factors (mean correction is
                # applied later, rank-1, onto the `a` psums) -----------
                with tc.tile_pool(name="psm", bufs=1, space="PSUM") as psm:
                    # ct copies start immediately at Cuu stop
                    nc.scalar.activation(
                        ct_bf[:, 0, :], cps0[:, 0:C], AF.Copy, scale=INV_N
                    )
                    nc.vector.tensor_scalar_mul(
                        ct_bf[:, 1, P:C], cps1[:, 0:P], INV_N
                    )
                    # su/N columns (for ws = su^T W' / N)
                    nc.vector.tensor_scalar_mul(
                        su_col[:, 0:1], cps0[:, C:C + 1], INV_N
                    )
                    nc.scalar.activation(
                        su_col[:, 1:2], cps1[:, C - P:C - P + 1], AF.Copy,
                        scale=INV_N,
                    )
                    # missing Ctilde block by symmetry
                    ctt = psm.tile([P, P], BF16, tag="ctt", name="ctt")
                    nc.tensor.transpose(ctt[:], ct_bf[:, 0, P:C], ident_bf[:])
                    nc.vector.tensor_copy(ct_bf[:, 1, 0:P], ctt[:])
                    # su as a row: su_nrow = -su  (scale (su/N) by -N)
                    su_rowT = psm.tile([1, C], F32, tag="surt", name="surt")
                    for t in range(2):
                        nc.tensor.matmul(
                            su_rowT[0:1, t * P:(t + 1) * P],
                            su_col[:, t:t + 1],
                            ident_bf[:],
                            start=True,
                            stop=True,
                        )
                    nc.scalar.activation(
                        su_nrow[:], su_rowT[:], AF.Copy, scale=-1.0
                    )

            # ---- phase 2: statistics / W_eff --------------------------
            with tc.tile_pool(name="sm", bufs=1) as sm:
                ak_bf = sm.tile([P, 2, HD], BF16)
                m_kv = sm.tile([P, 2, 2 * HD], BF16)
                ws_bf = sm.tile([1, 2 * HD], BF16)
                with tc.tile_pool(name="psA", bufs=1, space="PSUM") as psA:
                    # ws = su^T [Wk^T | Wv^T] / N   [1, 2hd]
                    wsp = psA.tile([1, 2 * HD], F32, tag="ws", name="ws")
                    for half in range(2):
                        for tp in range(2):
                            nc.tensor.matmul(
                                wsp[0:1, half * HD:(half + 1) * HD],
                                su_col[:, tp:tp + 1],
                                wkvT_bf[:, tp, half * HD:(half + 1) * HD],
                                start=(tp == 0),
                                stop=(tp == 1),
                            )
                    nc.scalar.copy(ws_bf[0:1, 0:HD], wsp[0:1, 0:HD])
                    nc.vector.tensor_copy(
                        ws_bf[0:1, HD:2 * HD], wsp[0:1, HD:2 * HD]
                    )
                    # a = C0 @ [Wk^T | Wv^T] - su (x) ws   [c, 2hd]
                    aps = []
                    for t in range(2):
                        ap_t = psA.tile([P, 2 * HD], F32, tag=f"a{t}", name=f"a{t}")
                        aps.append(ap_t)
                        for half in range(2):
                            for tp in range(2):
                                nc.tensor.matmul(
                                    ap_t[:, half * HD:(half + 1) * HD],
                                    ct_bf[:, tp, t * P:(t + 1) * P],
                                    wkvT_bf[:, tp, half * HD:(half + 1) * HD],
                                    start=(tp == 0),
                                    stop=(tp == 1),
                                )
                            nc.tensor.matmul(
                                ap_t[:, half * HD:(half + 1) * HD],
                                su_nrow[0:1, t * P:(t + 1) * P],
                                ws_bf[0:1, half * HD:(half + 1) * HD],
                                start=False,
                                stop=True,
                            )
                    # v-chain first (mask_rv gates the per-pair kv muls):
                    # m_v from psum on DVE, while ACT copies the k halves
                    vv = psA.tile([P, 8], F32, tag="vv", name="vv")
                    for t in range(2):
                        nc.vector.tensor_mul(
                            m_kv[:, t, HD:2 * HD], aps[t][:, HD:2 * HD],
                            wkvT_bf[:, t, HD:2 * HD],
                        )
                        nc.scalar.copy(ak_bf[:, t, :], aps[t][:, 0:HD])
                    for g in range(4):
                        for tp in range(2):
                            nc.tensor.matmul(
                                vv[:, 4 + g:4 + g + 1],
                                m_kv[:, tp, HD + g * P:HD + (g + 1) * P],
                                ones_bf[:],
                                start=(tp == 0),
                                stop=(tp == 1),
                            )
                    nc.scalar.activation(
                        rv_col[:], vv[:, 4:8], AF.Sqrt, bias=eps_col[:, 0:1]
                    )
                    nc.vector.reciprocal(rv_col[:], rv_col[:])
                    for jp in range(4):
                        nc.gpsimd.tensor_scalar_mul(
                            mask_rv[:, jp, :], mask_bf[:], rv_col[:, jp:jp + 1]
                        )
                    # k-chain: m_k from the bf16 copies (gpsimd, all-SBUF)
                    for t in range(2):
                        nc.gpsimd.tensor_mul(
                            m_kv[:, t, 0:HD], ak_bf[:, t, :], wkvT_bf[:, t, 0:HD]
                        )
                    for g in range(4):
                        for tp in range(2):
                            nc.tensor.matmul(
                                vv[:, g:g + 1],
                                m_kv[:, tp, g * P:(g + 1) * P],
                                ones_bf[:],
                                start=(tp == 0),
                                stop=(tp == 1),
                            )
                    nc.scalar.activation(
                        rk_col[:], vv[:, 0:4], AF.Sqrt, bias=eps_col[:, 0:1]
                    )
                    nc.vector.reciprocal(rk_col[:], rk_col[:])

                # per head-pair: kv^T block, mask*rv, bx*rk, W_eff accum
                with tc.tile_pool(name="psP", bufs=1, space="PSUM") as psP:
                    wps2 = [
                        psP.tile([P, OUT], F32, tag=f"weff{t}", name=f"wps{t}")
                        for t in range(2)
                    ]
                    for jp in range(4):
                        sl = slice(jp * P, (jp + 1) * P)
                        sd = psP.tile([P, P], F32, tag="sd", bufs=2, name="sd")
                        for tp in range(2):
                            nc.tensor.matmul(
                                sd[:],
                                wkvT_bf[:, tp, HD + jp * P:HD + (jp + 1) * P],
                                ak_bf[:, tp, sl],
                                start=(tp == 0),
                                stop=(tp == 1),
                            )
                        kv_bf = sm.tile([P, P], BF16, tag=f"kv{jp}", name=f"kv{jp}")
                        nc.vector.tensor_mul(kv_bf[:], sd[:], mask_rv[:, jp, :])
                        bx = psP.tile([P, OUT], F32, tag="bx", bufs=2, name="bx")
                        nc.tensor.matmul(
                            bx[:], kv_bf[:], woT_bf[:, jp, :], start=True, stop=True
                        )
                        bx_bf = sm.tile([P, OUT], BF16, tag=f"bxb{jp}", name=f"bxb{jp}")
                        if jp % 2 == 0:
                            nc.scalar.activation(
                                bx_bf[:], bx[:], AF.Copy,
                                scale=rk_col[:, jp:jp + 1],
                            )
                        else:
                            nc.vector.tensor_scalar_mul(
                                bx_bf[:], bx[:], rk_col[:, jp:jp + 1]
                            )
                        for t in range(2):
                            nc.tensor.matmul(
                                wps2[t][:],
                                wq_bf[:, jp, t * P:(t + 1) * P],
                                bx_bf[:],
                                start=(jp == 0),
                                stop=(jp == 3),
                            )
                    nc.scalar.copy(weff[:, 0, :], wps2[0][:])
                    nc.vector.tensor_copy(weff[:, 1, :], wps2[1][:])

            # ---- phase 3: out^T = W_eff^T u^T (bf16 PE stream) --------
            with (
                tc.tile_pool(name="opool", bufs=2) as opool,
                tc.tile_pool(name="pout", bufs=6, space="PSUM") as pout,
            ):
                for och in range(OCH):
                    osb = opool.tile([P, 2, CH_ROWS], BF16, tag="osb", name="osb")
                    for sg in range(NGROUPS // OCH):
                        s = och * (NGROUPS // OCH) + sg
                        for ob in range(2):
                            po = pout.tile([P, GROUP], F32, tag="po", name="po")
                            for t in range(2):
                                nc.tensor.matmul(
                                    po[:],
                                    weff[:, t, ob * P:(ob + 1) * P],
                                    uT[:, t, s * GROUP:(s + 1) * GROUP],
                                    start=(t == 0),
                                    stop=(t == 1),
                                )
                            dst = osb[:, ob, sg * GROUP:(sg + 1) * GROUP]
                            if ob == 0:
                                nc.vector.tensor_copy(dst, po[:])
                            else:
                                nc.scalar.copy(dst, po[:])
                        if sg % 2 == 1:
                            # store each 2-group stripe (both o-blocks)
                            h0 = (sg - 1) * GROUP
                            nc.sync.dma_start(
                                out_d[:, och * 2 * CH_ROWS:(och + 1) * 2 * CH_ROWS]
                                .rearrange("p (a n) -> p a n", a=2)[
                                    :, :, h0:h0 + 2 * GROUP
                                ],
                                osb[:, :, h0:h0 + 2 * GROUP],
                            )

    nc.compile()
    return nc


_NC_CACHE = None


def _get_nc():
    global _NC_CACHE
    if _NC_CACHE is None:
        _NC_CACHE = build_nc()
    return _NC_CACHE


def make_in_maps(u_src, Wq, Wk, Wv, Wo):
    """Per-core input dicts. Core c = (batch c//2, half c%2); its own
    half of the grid axis is permuted to the front of u.  Everything is
    cast to bf16 and packed partition-major host-side."""
    bf = ml_dtypes.bfloat16
    wq_b = np.ascontiguousarray(
        Wq.reshape(4, P, C).transpose(1, 0, 2).reshape(P, 4 * C).astype(bf)
    )
    wkv = np.concatenate([Wk.T, Wv.T], axis=1)           # [C, 2HD]
    wkv_b = np.ascontiguousarray(
        wkv.reshape(2, P, 2 * HD).transpose(1, 0, 2).reshape(P, 4 * HD).astype(bf)
    )
    wot_b = np.ascontiguousarray(
        Wo.T.reshape(4, P, OUT).transpose(1, 0, 2).reshape(P, 4 * OUT).astype(bf)
    )
    in_maps = []
    for c in range(8):
        b, half = c // 2, c % 2
        ub = u_src[b]
        mine = ub[half * N_HALF:(half + 1) * N_HALF]
        other = ub[(1 - half) * N_HALF:(2 - half) * N_HALF]
        u_perm = np.concatenate([mine, other], axis=0)   # [N_FULL, C]
        u_r = np.empty((P, N_CHUNKS, SUBT, C1), dtype=bf)
        u_r[:, :, :, 0:C] = u_perm.reshape(N_CHUNKS, P, SUBT, C).transpose(
            1, 0, 2, 3
        ).astype(bf)
        u_r[:, :, :, C] = bf(1.0)
        u_r = u_r.reshape(P, U_ROW)
        in_maps.append({"u": u_r, "wq": wq_b, "wkv": wkv_b, "wot": wot_b})
    return in_maps


def assemble_output(results, bo):
    """Device emits out_r [P, OCH, 2, 2048] bf16 where element
    (p, och, a, j*128+pc) = out[row och*2048 + pc*16 + j, o=a*128+p]."""
    out = np.empty((4, N_FULL, OUT), dtype=np.float32)
    for c in range(8):
        b, half = c // 2, c % 2
        a = np.asarray(results[c]["out"]).astype(np.float32)
        a = a.reshape(P, OCH, 2, SUBT, P)        # [p, och, a, j, pc]
        a = a.transpose(1, 4, 3, 2, 0).reshape(N_HALF, OUT)
        out[b, half * N_HALF:(half + 1) * N_HALF] = a
    if np.any(bo):
        out += bo.reshape(1, 1, OUT)
    return out


def run(inputs, trace=False, tmpdir=None):
    """inputs: dict as from reference.setup_inputs(). Returns
    (full_output, BassKernelResults)."""
    u_src = np.asarray(inputs["u_src"], dtype=np.float32)
    Wq = np.asarray(inputs["Wq"], dtype=np.float32)
    Wk = np.asarray(inputs["Wk"], dtype=np.float32)
    Wv = np.asarray(inputs["Wv"], dtype=np.float32)
    Wo = np.asarray(inputs["Wo"], dtype=np.float32)
    bo = np.asarray(inputs["bo"], dtype=np.float32)
    nc = _get_nc()
    in_maps = make_in_maps(u_src, Wq, Wk, Wv, Wo)
    res = run_bass_kernel_spmd(
        nc, in_maps, core_ids=list(range(8)), trace=trace, tmpdir=tmpdir
    )
    return assemble_output(res.results, bo), res


def kernel(**inputs):
    out, _ = run(inputs, trace=False)
    return out


# revision 27
# speedup vs baseline: 1.2802x; 1.2802x over previous
"""Trainium2 Bass kernel for nn_AttentionKernelIntegral (linear attention
with instance-normed k/v, collapsed algebraically).

Math
----
Reference computes (per batch, H=8 heads, D=64, C=OUT=256, N=16384):
    q = u @ Wq^T ; k = u @ Wk^T ; v = u @ Wv^T          (per head blocks)
    khat = instnorm_n(k); vhat = instnorm_n(v)
    kv_h = (1/N) khat_h^T vhat_h                        [D, D]
    out  = concat_h(q_h @ kv_h) @ Wo^T + bo

Everything downstream of u is linear except the instance-norm statistics
(exact functions of first/second moments over n), so the network
collapses to   out = u @ W_eff + bo.  With the *centered* covariance

    Ctilde = (Cuu - su su^T / N) / N,   Cuu = u^T u, su = u^T 1

the means drop out entirely:

    kv_h   = Dk_h (Wk_h Ctilde Wv_h^T) Dv_h
    vark_d = (Wk Ctilde Wk^T)_dd ;  Dk = diag(rsqrt(vark + eps))
    W_eff  = sum_h Wq_h^T kv_h Wo_h^T                   [C, OUT]

Sharding: 8 cores = 4 batches x 2 grid-halves.  Each core receives the
full u for its batch (bf16, with ITS half permuted first), accumulates
Cuu over the full grid, and emits out^T for its own half.

Layouts: the host pre-packs u / weights / output DRAM tensors
partition-major so every DMA descriptor moves 2-8 KB contiguous per
partition.  u and weights are bf16 (host cast); output is stored bf16
(out^T) and upcast + unpermuted on the host.

Cuu uses symmetry: the row-block-1 matmul streams only cols 128..256;
the missing [128,128] block of Ctilde is reconstructed by one PE
transpose.  The -su su^T/N correction is accumulated straight onto the
Cuu PSUM banks by two K=1 matmuls.  Variances are produced directly in
column format (N=1 matmuls against a ones column); rv is folded into
the per-pair block-diag mask and rk into the bx copy, so no scaled
weight copies are needed.
"""

import numpy as np
import ml_dtypes

import concourse.bass as bass
import concourse.tile as tile
from concourse import bacc, mybir
from concourse.bass_utils import run_bass_kernel_spmd
from concourse.masks import make_identity, make_block_diagonal

F32 = mybir.dt.float32
BF16 = mybir.dt.bfloat16
AL = mybir.AluOpType
AF = mybir.ActivationFunctionType

P = 128
N_FULL = 16384
N_HALF = 8192
C = 256
HD = 512          # H * D
OUT = 256
EPS = 1e-5
CH_ROWS = 2048
N_CHUNKS = N_FULL // CH_ROWS      # 8 chunks of 2048 rows (full grid)
SUBT = CH_ROWS // P               # 16 row-subtiles per chunk
G_ALL = N_FULL // P               # 128 row-tiles total
G_MINE = N_HALF // P              # first 64 belong to this core
INV_N = 1.0 / float(N_FULL)
GROUP = 512                       # phase-3 column group of out^T
NGROUPS = N_HALF // GROUP         # 16
OCH = 4                           # phase-3 store chunks (4 groups each)
C1 = C + 1                        # u row + embedded 1.0 (ones column)
U_ROW = N_CHUNKS * SUBT * C1      # per-partition elements of u_r
O_ROW = OCH * 2 * CH_ROWS         # 16384 per-partition elements of out_r


def build_nc():
    nc = bacc.Bacc(
        "TRN2",
        target_bir_lowering=False,
        debug=False,
        num_devices=8,
    )
    u_d = nc.dram_tensor("u", [P, U_ROW], BF16, kind="ExternalInput").ap()
    wq_d = nc.dram_tensor("wq", [P, 4 * C], BF16, kind="ExternalInput").ap()
    wkv_d = nc.dram_tensor("wkv", [P, 2 * 2 * HD], BF16, kind="ExternalInput").ap()
    wot_d = nc.dram_tensor("wot", [P, 4 * OUT], BF16, kind="ExternalInput").ap()
    out_d = nc.dram_tensor("out", [P, O_ROW], BF16, kind="ExternalOutput").ap()

    with tile.TileContext(nc) as tc:
        with tc.tile_pool(name="pers", bufs=1) as pers:
            # ---- persistent tiles -------------------------------------
            uT = pers.tile([P, 2, N_HALF], BF16)         # u^T (bf16, own half)
            ident = pers.tile([P, P], F32)
            make_identity(nc, ident[:])
            ident_bf = pers.tile([P, P], BF16)
            nc.vector.tensor_copy(ident_bf[:], ident[:])
            mask_f = pers.tile([P, P], F32)
            make_block_diagonal(nc, mask_f[:], 64)
            mask_bf = pers.tile([P, P], BF16)
            nc.vector.tensor_copy(mask_bf[:], mask_f[:])
            wq_bf = pers.tile([P, 4, C], BF16)           # Wq natural [hd, c]
            wkvT_bf = pers.tile([P, 2, 2 * HD], BF16)    # [Wk^T | Wv^T] [c, 2hd]
            woT_bf = pers.tile([P, 4, OUT], BF16)        # Wo^T  [hd, o]
            weff = pers.tile([P, 2, OUT], BF16)
            ct_bf = pers.tile([P, 2, C], BF16)           # Ctilde (bf16)
            ones_bf = pers.tile([P, 1], BF16)
            nc.vector.memset(ones_bf[:], 1.0)
            eps_col = pers.tile([P, 1], F32)
            nc.vector.memset(eps_col[:], EPS)
            su_col = pers.tile([P, 2], BF16)             # su / N
            su_nrow = pers.tile([1, C], BF16)            # -su / N
            rk_col = pers.tile([P, 4], F32)
            rv_col = pers.tile([P, 4], F32)
            mask_rv = pers.tile([P, 4, P], BF16)         # mask * rv (per pair)
            # prewarm ACT tables used later (Copy via scalar.mul, Sqrt)
            warm = pers.tile([1, 8], F32)
            nc.vector.memset(warm[:], 1.0)
            nc.scalar.mul(warm[:], warm[:], 1.0)
            nc.scalar.activation(warm[:], warm[:], AF.Sqrt)

            # ---- phase 1: stream u, accumulate Cuu, transpose own half
            with (
                tc.tile_pool(name="upool", bufs=3) as upool,
                tc.tile_pool(name="pacc", bufs=1, space="PSUM") as pacc,
                tc.tile_pool(name="ptr", bufs=3, space="PSUM") as ptr,
            ):
                cps0 = pacc.tile([P, C + 1], F32, tag="c0", name="c0")
                cps1 = pacc.tile([P, C + 1 - P], F32, tag="c1", name="c1")
                # chunk 0 arrives in three j-slices so the PE starts early
                sched = [(0, 0, 4), (0, 4, 4), (0, 8, 8)]
                for ch in range(1, N_CHUNKS):
                    sched.append((ch, 0, SUBT))
                ub = None
                first_dmas = 0
                for ch, j0, nsub in sched:
                    if j0 == 0:
                        ub = upool.tile([P, SUBT, C1], BF16, tag="ub", name="ub")
                    src_ap = u_d[:, ch * SUBT * C1:(ch + 1) * SUBT * C1].rearrange(
                        "p (j c) -> p j c", c=C1
                    )
                    nc.sync.dma_start(
                        ub[:, j0:j0 + nsub, :], src_ap[:, j0:j0 + nsub, :]
                    )
                    if (ch, j0) == (N_CHUNKS - 1, 0):
                        # weights are phase-2 inputs; issue their DMAs
                        # after the u stream so they don't delay it
                        nc.sync.dma_start(
                            wq_bf[:], wq_d.rearrange("p (a c) -> p a c", c=C)
                        )
                        nc.sync.dma_start(
                            wkvT_bf[:],
                            wkv_d.rearrange("p (a c) -> p a c", c=2 * HD),
                        )
                        nc.sync.dma_start(
                            woT_bf[:], wot_d.rearrange("p (a c) -> p a c", c=OUT)
                        )
                    for j in range(j0, j0 + nsub):
                        g = ch * SUBT + j
                        nc.tensor.matmul(
                            cps0[:],
                            ub[:, j, 0:P],
                            ub[:, j, :],
                            start=(g == 0),
                            stop=(g == G_ALL - 1),
                        )
                        nc.tensor.matmul(
                            cps1[:],
                            ub[:, j, P:C],
                            ub[:, j, P:C + 1],
                            start=(g == 0),
                            stop=(g == G_ALL - 1),
                        )
                        if g < G_MINE:
                            tps = ptr.tile([P, C], BF16, tag="uT", name="tps")
                            for t in range(2):
                                nc.tensor.transpose(
                                    tps[:, t * P:(t + 1) * P],
                                    ub[:, j, t * P:(t + 1) * P],
                                    ident_bf[:],
                                )
                            if g % 2 == 0:
                                nc.vector.tensor_copy(
                                    uT[:, :, g * P:(g + 1) * P],
                                    tps[:].rearrange("p (t n) -> p t n", t=2),
                                )
                            else:
                                nc.scalar.copy(
                                    uT[:, :, g * P:(g + 1) * P],
                                    tps[:].rearrange("p (t n) -> p t n", t=2),
                                )

                # ---- C0 = Cuu/N (bf16) + su factors (mean correction is
                # applied later, rank-1, onto the `a` psums) -----------
                with tc.tile_pool(name="psm", bufs=1, space="PSUM") as psm:
                    # ct copies start immediately at Cuu stop
                    nc.scalar.activation(
                        ct_bf[:, 0, :], cps0[:, 0:C], AF.Copy, scale=INV_N
                    )
                    nc.vector.tensor_scalar_mul(
                        ct_bf[:, 1, P:C], cps1[:, 0:P], INV_N
                    )
                    # su/N columns (for ws = su^T W' / N)
                    nc.vector.tensor_scalar_mul(
                        su_col[:, 0:1], cps0[:, C:C + 1], INV_N
                    )
                    nc.scalar.activation(
                        su_col[:, 1:2], cps1[:, C - P:C - P + 1], AF.Copy,
                        scale=INV_N,
                    )
                    # missing Ctilde block by symmetry
                    ctt = psm.tile([P, P], BF16, tag="ctt", name="ctt")
                    nc.tensor.transpose(ctt[:], ct_bf[:, 0, P:C], ident_bf[:])
                    nc.vector.tensor_copy(ct_bf[:, 1, 0:P], ctt[:])
                    # su as a row: su_nrow = -su  (scale (su/N) by -N)
                    su_rowT = psm.tile([1, C], F32, tag="surt", name="surt")
                    for t in range(2):
                        nc.tensor.matmul(
                            su_rowT[0:1, t * P:(t + 1) * P],
                            su_col[:, t:t + 1],
                            ident_bf[:],
                            start=True,
                            stop=True,
                        )
                    nc.scalar.activation(
                        su_nrow[:], su_rowT[:], AF.Copy, scale=-1.0
                    )

            # ---- phase 2: statistics / W_eff --------------------------
            with tc.tile_pool(name="sm", bufs=1) as sm:
                ak_bf = sm.tile([P, 2, HD], BF16)
                m_kv = sm.tile([P, 2, 2 * HD], BF16)
                ws_bf = sm.tile([1, 2 * HD], BF16)
                with tc.tile_pool(name="psA", bufs=1, space="PSUM") as psA:
                    # ws = su^T [Wk^T | Wv^T] / N   [1, 2hd]
                    wsp = psA.tile([1, 2 * HD], F32, tag="ws", name="ws")
                    for half in range(2):
                        for tp in range(2):
                            nc.tensor.matmul(
                                wsp[0:1, half * HD:(half + 1) * HD],
                                su_col[:, tp:tp + 1],
                                wkvT_bf[:, tp, half * HD:(half + 1) * HD],
                                start=(tp == 0),
                                stop=(tp == 1),
                            )
                    nc.scalar.copy(ws_bf[0:1, 0:HD], wsp[0:1, 0:HD])
                    nc.vector.tensor_copy(
                        ws_bf[0:1, HD:2 * HD], wsp[0:1, HD:2 * HD]
                    )
                    # a = C0 @ [Wk^T | Wv^T] - su (x) ws   [c, 2hd]
                    aps = []
                    for t in range(2):
                        ap_t = psA.tile([P, 2 * HD], F32, tag=f"a{t}", name=f"a{t}")
                        aps.append(ap_t)
                        for half in range(2):
                            for tp in range(2):
                                nc.tensor.matmul(
                                    ap_t[:, half * HD:(half + 1) * HD],
                                    ct_bf[:, tp, t * P:(t + 1) * P],
                                    wkvT_bf[:, tp, half * HD:(half + 1) * HD],
                                    start=(tp == 0),
                                    stop=(tp == 1),
                                )
                            nc.tensor.matmul(
                                ap_t[:, half * HD:(half + 1) * HD],
                                su_nrow[0:1, t * P:(t + 1) * P],
                                ws_bf[0:1, half * HD:(half + 1) * HD],
                                start=False,
                                stop=True,
                            )
                    # v-chain first (mask_rv gates the per-pair kv muls):
                    # m_v from psum on DVE, while ACT copies the k halves
                    vv = psA.tile([P, 8], F32, tag="vv", name="vv")
                    for t in range(2):
                        nc.vector.tensor_mul(
                            m_kv[:, t, HD:2 * HD], aps[t][:, HD:2 * HD],
                            wkvT_bf[:, t, HD:2 * HD],
                        )
                        nc.scalar.copy(ak_bf[:, t, :], aps[t][:, 0:HD])
                    for g in range(4):
                        for tp in range(2):
                            nc.tensor.matmul(
                                vv[:, 4 + g:4 + g + 1],
                                m_kv[:, tp, HD + g * P:HD + (g + 1) * P],
                                ones_bf[:],
                                start=(tp == 0),
                                stop=(tp == 1),
                            )
                    nc.scalar.activation(
                        rv_col[:], vv[:, 4:8], AF.Sqrt, bias=eps_col[:, 0:1]
                    )
                    nc.vector.reciprocal(rv_col[:], rv_col[:])
                    for jp in range(4):
                        nc.vector.tensor_scalar_mul(
                            mask_rv[:, jp, :], mask_bf[:], rv_col[:, jp:jp + 1]
                        )
                    # k-chain: m_k from the bf16 copies (2x DVE rate)
                    for t in range(2):
                        nc.vector.tensor_mul(
                            m_kv[:, t, 0:HD], ak_bf[:, t, :], wkvT_bf[:, t, 0:HD]
                        )
                    for g in range(4):
                        for tp in range(2):
                            nc.tensor.matmul(
                                vv[:, g:g + 1],
                                m_kv[:, tp, g * P:(g + 1) * P],
                                ones_bf[:],
                                start=(tp == 0),
                                stop=(tp == 1),
                            )
                    nc.scalar.activation(
                        rk_col[:], vv[:, 0:4], AF.Sqrt, bias=eps_col[:, 0:1]
                    )
                    nc.vector.reciprocal(rk_col[:], rk_col[:])

                # per head-pair: kv^T block, mask*rv, bx*rk, W_eff accum
                with tc.tile_pool(name="psP", bufs=1, space="PSUM") as psP:
                    wps2 = [
                        psP.tile([P, OUT], F32, tag=f"weff{t}", name=f"wps{t}")
                        for t in range(2)
                    ]
                    for jp in range(4):
                        sl = slice(jp * P, (jp + 1) * P)
                        sd = psP.tile([P, P], F32, tag="sd", bufs=2, name="sd")
                        for tp in range(2):
                            nc.tensor.matmul(
                                sd[:],
                                wkvT_bf[:, tp, HD + jp * P:HD + (jp + 1) * P],
                                ak_bf[:, tp, sl],
                                start=(tp == 0),
                                stop=(tp == 1),
                            )
                        kv_bf = sm.tile([P, P], BF16, tag=f"kv{jp}", name=f"kv{jp}")
                        nc.vector.tensor_mul(kv_bf[:], sd[:], mask_rv[:, jp, :])
                        bx = psP.tile([P, OUT], F32, tag="bx", bufs=2, name="bx")
                        nc.tensor.matmul(
                            bx[:], kv_bf[:], woT_bf[:, jp, :], start=True, stop=True
                        )
                        bx_bf = sm.tile([P, OUT], BF16, tag=f"bxb{jp}", name=f"bxb{jp}")
                        if jp % 2 == 0:
                            nc.scalar.activation(
                                bx_bf[:], bx[:], AF.Copy,
                                scale=rk_col[:, jp:jp + 1],
                            )
                        else:
                            nc.vector.tensor_scalar_mul(
                                bx_bf[:], bx[:], rk_col[:, jp:jp + 1]
                            )
                        for t in range(2):
                            nc.tensor.matmul(
                                wps2[t][:],
                                wq_bf[:, jp, t * P:(t + 1) * P],
                                bx_bf[:],
                                start=(jp == 0),
                                stop=(jp == 3),
                            )
                    nc.scalar.copy(weff[:, 0, :], wps2[0][:])
                    nc.vector.tensor_copy(weff[:, 1, :], wps2[1][:])

            # ---- phase 3: out^T = W_eff^T u^T (bf16 PE stream) --------
            with (
                tc.tile_pool(name="opool", bufs=2) as opool,
                tc.tile_pool(name="pout", bufs=6, space="PSUM") as pout,
            ):
                for och in range(OCH):
                    osb = opool.tile([P, 2, CH_ROWS], BF16, tag="osb", name="osb")
                    for sg in range(NGROUPS // OCH):
                        s = och * (NGROUPS // OCH) + sg
                        for ob in range(2):
                            po = pout.tile([P, GROUP], F32, tag="po", name="po")
                            for t in range(2):
                                nc.tensor.matmul(
                                    po[:],
                                    weff[:, t, ob * P:(ob + 1) * P],
                                    uT[:, t, s * GROUP:(s + 1) * GROUP],
                                    start=(t == 0),
                                    stop=(t == 1),
                                )
                            dst = osb[:, ob, sg * GROUP:(sg + 1) * GROUP]
                            if ob == 0:
                                nc.vector.tensor_copy(dst, po[:])
                            else:
                                nc.scalar.copy(dst, po[:])
                        if sg % 2 == 1:
                            # store each 2-group stripe (both o-blocks)
                            h0 = (sg - 1) * GROUP
                            nc.sync.dma_start(
                                out_d[:, och * 2 * CH_ROWS:(och + 1) * 2 * CH_ROWS]
                                .rearrange("p (a n) -> p a n", a=2)[
                                    :, :, h0:h0 + 2 * GROUP
                                ],
                                osb[:, :, h0:h0 + 2 * GROUP],
                            )

    nc.compile()
    return nc


_NC_CACHE = None


def _get_nc():
    global _NC_CACHE
    if _NC_CACHE is None:
        _NC_CACHE = build_nc()
    return _NC_CACHE


def make_in_maps(u_src, Wq, Wk, Wv, Wo):
    """Per-core input dicts. Core c = (batch c//2, half c%2); its own
    half of the grid axis is permuted to the front of u.  Everything is
    cast to bf16 and packed partition-major host-side."""
    bf = ml_dtypes.bfloat16
    wq_b = np.ascontiguousarray(
        Wq.reshape(4, P, C).transpose(1, 0, 2).reshape(P, 4 * C).astype(bf)
    )
    wkv = np.concatenate([Wk.T, Wv.T], axis=1)           # [C, 2HD]
    wkv_b = np.ascontiguousarray(
        wkv.reshape(2, P, 2 * HD).transpose(1, 0, 2).reshape(P, 4 * HD).astype(bf)
    )
    wot_b = np.ascontiguousarray(
        Wo.T.reshape(4, P, OUT).transpose(1, 0, 2).reshape(P, 4 * OUT).astype(bf)
    )
    in_maps = []
    for c in range(8):
        b, half = c // 2, c % 2
        ub = u_src[b]
        mine = ub[half * N_HALF:(half + 1) * N_HALF]
        other = ub[(1 - half) * N_HALF:(2 - half) * N_HALF]
        u_perm = np.concatenate([mine, other], axis=0)   # [N_FULL, C]
        u_r = np.empty((P, N_CHUNKS, SUBT, C1), dtype=bf)
        u_r[:, :, :, 0:C] = u_perm.reshape(N_CHUNKS, P, SUBT, C).transpose(
            1, 0, 2, 3
        ).astype(bf)
        u_r[:, :, :, C] = bf(1.0)
        u_r = u_r.reshape(P, U_ROW)
        in_maps.append({"u": u_r, "wq": wq_b, "wkv": wkv_b, "wot": wot_b})
    return in_maps


def assemble_output(results, bo):
    """Device emits out_r [P, OCH, 2, 2048] bf16 where element
    (p, och, a, j*128+pc) = out[row och*2048 + pc*16 + j, o=a*128+p]."""
    out = np.empty((4, N_FULL, OUT), dtype=np.float32)
    for c in range(8):
        b, half = c // 2, c % 2
        a = np.asarray(results[c]["out"]).astype(np.float32)
        a = a.reshape(P, OCH, 2, SUBT, P)        # [p, och, a, j, pc]
        a = a.transpose(1, 4, 3, 2, 0).reshape(N_HALF, OUT)
        out[b, half * N_HALF:(half + 1) * N_HALF] = a
    if np.any(bo):
        out += bo.reshape(1, 1, OUT)
    return out


def run(inputs, trace=False, tmpdir=None):
    """inputs: dict as from reference.setup_inputs(). Returns
    (full_output, BassKernelResults)."""
    u_src = np.asarray(inputs["u_src"], dtype=np.float32)
    Wq = np.asarray(inputs["Wq"], dtype=np.float32)
    Wk = np.asarray(inputs["Wk"], dtype=np.float32)
    Wv = np.asarray(inputs["Wv"], dtype=np.float32)
    Wo = np.asarray(inputs["Wo"], dtype=np.float32)
    bo = np.asarray(inputs["bo"], dtype=np.float32)
    nc = _get_nc()
    in_maps = make_in_maps(u_src, Wq, Wk, Wv, Wo)
    res = run_bass_kernel_spmd(
        nc, in_maps, core_ids=list(range(8)), trace=trace, tmpdir=tmpdir
    )
    return assemble_output(res.results, bo), res


def kernel(**inputs):
    out, _ = run(inputs, trace=False)
    return out


# revision 33
# speedup vs baseline: 1.3081x; 1.0218x over previous
"""Trainium2 Bass kernel for nn_AttentionKernelIntegral (linear attention
with instance-normed k/v, collapsed algebraically).

Math
----
Reference computes (per batch, H=8 heads, D=64, C=OUT=256, N=16384):
    q = u @ Wq^T ; k = u @ Wk^T ; v = u @ Wv^T          (per head blocks)
    khat = instnorm_n(k); vhat = instnorm_n(v)
    kv_h = (1/N) khat_h^T vhat_h                        [D, D]
    out  = concat_h(q_h @ kv_h) @ Wo^T + bo

Everything downstream of u is linear except the instance-norm statistics
(exact functions of first/second moments over n), so the network
collapses to   out = u @ W_eff + bo.  With the *centered* covariance

    Ctilde = (Cuu - su su^T / N) / N,   Cuu = u^T u, su = u^T 1

the means drop out entirely:

    kv_h   = Dk_h (Wk_h Ctilde Wv_h^T) Dv_h
    vark_d = (Wk Ctilde Wk^T)_dd ;  Dk = diag(rsqrt(vark + eps))
    W_eff  = sum_h Wq_h^T kv_h Wo_h^T                   [C, OUT]

Sharding: 8 cores = 4 batches x 2 grid-halves.  Each core receives the
full u for its batch (bf16, with ITS half permuted first), accumulates
Cuu over the full grid, and emits out^T for its own half.

Layouts: the host pre-packs u / weights / output DRAM tensors
partition-major so every DMA descriptor moves 2-8 KB contiguous per
partition.  u and weights are bf16 (host cast); output is stored bf16
(out^T) and upcast + unpermuted on the host.

Cuu uses symmetry: the row-block-1 matmul streams only cols 128..256;
the missing [128,128] block of Ctilde is reconstructed by one PE
transpose.  The -su su^T/N correction is accumulated straight onto the
Cuu PSUM banks by two K=1 matmuls.  Variances are produced directly in
column format (N=1 matmuls against a ones column); rv is folded into
the per-pair block-diag mask and rk into the bx copy, so no scaled
weight copies are needed.
"""

import numpy as np
import ml_dtypes

import concourse.bass as bass
import concourse.tile as tile
from concourse import bacc, mybir
from concourse.bass_utils import run_bass_kernel_spmd
from concourse.masks import make_identity, make_block_diagonal

F32 = mybir.dt.float32
BF16 = mybir.dt.bfloat16
AL = mybir.AluOpType
AF = mybir.ActivationFunctionType

P = 128
N_FULL = 16384
N_HALF = 8192
C = 256
HD = 512          # H * D
OUT = 256
EPS = 1e-5
CH_ROWS = 2048
N_CHUNKS = N_FULL // CH_ROWS      # 8 chunks of 2048 rows (full grid)
SUBT = CH_ROWS // P               # 16 row-subtiles per chunk
G_ALL = N_FULL // P               # 128 row-tiles total
G_MINE = N_HALF // P              # first 64 belong to this core
INV_N = 1.0 / float(N_FULL)
GROUP = 512                       # phase-3 column group of out^T
NGROUPS = N_HALF // GROUP         # 16
OCH = 4                           # phase-3 store chunks (4 groups each)
C1 = C + 1                        # u row + embedded 1.0 (ones column)
U_ROW = N_CHUNKS * SUBT * C1      # per-partition elements of u_r
O_ROW = OCH * 2 * CH_ROWS         # 16384 per-partition elements of out_r


def build_nc():
    nc = bacc.Bacc(
        "TRN2",
        target_bir_lowering=False,
        debug=False,
        num_devices=8,
    )
    u_d = nc.dram_tensor("u", [P, U_ROW], BF16, kind="ExternalInput").ap()
    wq_d = nc.dram_tensor("wq", [P, 4 * C], BF16, kind="ExternalInput").ap()
    wkv_d = nc.dram_tensor("wkv", [P, 2 * 2 * HD], BF16, kind="ExternalInput").ap()
    wot_d = nc.dram_tensor("wot", [P, 4 * OUT], BF16, kind="ExternalInput").ap()
    out_d = nc.dram_tensor("out", [P, O_ROW], BF16, kind="ExternalOutput").ap()

    with tile.TileContext(nc) as tc:
        with tc.tile_pool(name="pers", bufs=1) as pers:
            # ---- persistent tiles -------------------------------------
            uT = pers.tile([P, 2, N_HALF], BF16)         # u^T (bf16, own half)
            ident = pers.tile([P, P], F32)
            make_identity(nc, ident[:])
            ident_bf = pers.tile([P, P], BF16)
            nc.vector.tensor_copy(ident_bf[:], ident[:])
            mask_f = pers.tile([P, P], F32)
            make_block_diagonal(nc, mask_f[:], 64)
            mask_bf = pers.tile([P, P], BF16)
            nc.vector.tensor_copy(mask_bf[:], mask_f[:])
            wq_bf = pers.tile([P, 4, C], BF16)           # Wq natural [hd, c]
            wkvT_bf = pers.tile([P, 2, 2 * HD], BF16)    # [Wk^T | Wv^T] [c, 2hd]
            woT_bf = pers.tile([P, 4, OUT], BF16)        # Wo^T  [hd, o]
            weff = pers.tile([P, 2, OUT], BF16)
            ct_bf = pers.tile([P, 2, C], BF16)           # Ctilde (bf16)
            ones_bf = pers.tile([P, 1], BF16)
            nc.vector.memset(ones_bf[:], 1.0)
            eps_col = pers.tile([P, 1], F32)
            nc.vector.memset(eps_col[:], EPS)
            su_col = pers.tile([P, 2], BF16)             # su / N
            su_nrow = pers.tile([1, C], BF16)            # -su / N
            rk_col = pers.tile([P, 4], F32)
            rv_col = pers.tile([P, 4], F32)
            mask_rv = pers.tile([P, 4, P], BF16)         # mask * rv (per pair)
            # prewarm ACT tables used later (Copy via scalar.mul, Sqrt)
            warm = pers.tile([1, 8], F32)
            nc.vector.memset(warm[:], 1.0)
            nc.scalar.mul(warm[:], warm[:], 1.0)
            nc.scalar.activation(warm[:], warm[:], AF.Sqrt)

            # ---- phase 1: stream u, accumulate Cuu, transpose own half
            with (
                tc.tile_pool(name="upool", bufs=3) as upool,
                tc.tile_pool(name="pacc", bufs=1, space="PSUM") as pacc,
                tc.tile_pool(name="ptr", bufs=3, space="PSUM") as ptr,
            ):
                cps0 = pacc.tile([P, C + 1], F32, tag="c0", name="c0")
                cps1 = pacc.tile([P, C + 1 - P], F32, tag="c1", name="c1")
                # chunk 0 arrives in four j-slices so the PE starts early
                sched = [(0, 0, 2), (0, 2, 2), (0, 4, 4), (0, 8, 8)]
                for ch in range(1, N_CHUNKS):
                    sched.append((ch, 0, SUBT))
                ub = None
                first_dmas = 0
                for ch, j0, nsub in sched:
                    if j0 == 0:
                        ub = upool.tile([P, SUBT, C1], BF16, tag="ub", name="ub")
                    src_ap = u_d[:, ch * SUBT * C1:(ch + 1) * SUBT * C1].rearrange(
                        "p (j c) -> p j c", c=C1
                    )
                    nc.sync.dma_start(
                        ub[:, j0:j0 + nsub, :], src_ap[:, j0:j0 + nsub, :]
                    )
                    if (ch, j0) == (N_CHUNKS - 1, 0):
                        # weights are phase-2 inputs; issue their DMAs
                        # after the u stream so they don't delay it
                        nc.sync.dma_start(
                            wq_bf[:], wq_d.rearrange("p (a c) -> p a c", c=C)
                        )
                        nc.sync.dma_start(
                            wkvT_bf[:],
                            wkv_d.rearrange("p (a c) -> p a c", c=2 * HD),
                        )
                        nc.sync.dma_start(
                            woT_bf[:], wot_d.rearrange("p (a c) -> p a c", c=OUT)
                        )
                    for j in range(j0, j0 + nsub):
                        g = ch * SUBT + j
                        nc.tensor.matmul(
                            cps0[:],
                            ub[:, j, 0:P],
                            ub[:, j, :],
                            start=(g == 0),
                            stop=(g == G_ALL - 1),
                        )
                        nc.tensor.matmul(
                            cps1[:],
                            ub[:, j, P:C],
                            ub[:, j, P:C + 1],
                            start=(g == 0),
                            stop=(g == G_ALL - 1),
                        )
                        if g < G_MINE:
                            tps = ptr.tile([P, C], BF16, tag="uT", name="tps")
                            for t in range(2):
                                nc.tensor.transpose(
                                    tps[:, t * P:(t + 1) * P],
                                    ub[:, j, t * P:(t + 1) * P],
                                    ident_bf[:],
                                )
                            if g % 2 == 0:
                                nc.vector.tensor_copy(
                                    uT[:, :, g * P:(g + 1) * P],
                                    tps[:].rearrange("p (t n) -> p t n", t=2),
                                )
                            else:
                                nc.scalar.copy(
                                    uT[:, :, g * P:(g + 1) * P],
                                    tps[:].rearrange("p (t n) -> p t n", t=2),
                                )

                # ---- C0 = Cuu/N (bf16) + su factors (mean correction is
                # applied later, rank-1, onto the `a` psums) -----------
                with tc.tile_pool(name="psm", bufs=1, space="PSUM") as psm:
                    # ct copies start immediately at Cuu stop
                    nc.scalar.activation(
                        ct_bf[:, 0, :], cps0[:, 0:C], AF.Copy, scale=INV_N
                    )
                    nc.vector.tensor_scalar_mul(
                        ct_bf[:, 1, P:C], cps1[:, 0:P], INV_N
                    )
                    # su/N columns (for ws = su^T W' / N)
                    nc.vector.tensor_scalar_mul(
                        su_col[:, 0:1], cps0[:, C:C + 1], INV_N
                    )
                    nc.scalar.activation(
                        su_col[:, 1:2], cps1[:, C - P:C - P + 1], AF.Copy,
                        scale=INV_N,
                    )
                    # missing Ctilde block by symmetry
                    ctt = psm.tile([P, P], BF16, tag="ctt", name="ctt")
                    nc.tensor.transpose(ctt[:], ct_bf[:, 0, P:C], ident_bf[:])
                    nc.vector.tensor_copy(ct_bf[:, 1, 0:P], ctt[:])
                    # su as a row: su_nrow = -su  (scale (su/N) by -N)
                    su_rowT = psm.tile([1, C], F32, tag="surt", name="surt")
                    for t in range(2):
                        nc.tensor.matmul(
                            su_rowT[0:1, t * P:(t + 1) * P],
                            su_col[:, t:t + 1],
                            ident_bf[:],
                            start=True,
                            stop=True,
                        )
                    nc.vector.tensor_scalar_mul(su_nrow[:], su_rowT[:], -1.0)

            # ---- phase 2: statistics / W_eff --------------------------
            with tc.tile_pool(name="sm", bufs=1) as sm:
                ak_bf = sm.tile([P, 2, HD], BF16)
                m_kv = sm.tile([P, 2, 2 * HD], BF16)
                ws_bf = sm.tile([1, 2 * HD], BF16)
                with tc.tile_pool(name="psA", bufs=1, space="PSUM") as psA:
                    # ws = su^T [Wk^T | Wv^T] / N   [1, 2hd]
                    wsp = psA.tile([1, 2 * HD], F32, tag="ws", name="ws")
                    for half in range(2):
                        for tp in range(2):
                            nc.tensor.matmul(
                                wsp[0:1, half * HD:(half + 1) * HD],
                                su_col[:, tp:tp + 1],
                                wkvT_bf[:, tp, half * HD:(half + 1) * HD],
                                start=(tp == 0),
                                stop=(tp == 1),
                            )
                    nc.scalar.copy(ws_bf[0:1, 0:HD], wsp[0:1, 0:HD])
                    nc.vector.tensor_copy(
                        ws_bf[0:1, HD:2 * HD], wsp[0:1, HD:2 * HD]
                    )
                    # a = C0 @ [Wk^T | Wv^T] - su (x) ws   [c, 2hd]
                    # separate psum tiles per (t, half) so each consumer
                    # unblocks as soon as its own half (incl. fix) lands;
                    # v halves first — they gate the longer rv chain
                    aps = {}
                    for t in range(2):
                        for half in (1, 0):
                            ap_t = psA.tile(
                                [P, HD], F32, tag=f"a{t}{half}", name=f"a{t}{half}"
                            )
                            aps[(t, half)] = ap_t
                            for tp in range(2):
                                nc.tensor.matmul(
                                    ap_t[:],
                                    ct_bf[:, tp, t * P:(t + 1) * P],
                                    wkvT_bf[:, tp, half * HD:(half + 1) * HD],
                                    start=(tp == 0),
                                    stop=(tp == 1),
                                )
                            nc.tensor.matmul(
                                ap_t[:],
                                su_nrow[0:1, t * P:(t + 1) * P],
                                ws_bf[0:1, half * HD:(half + 1) * HD],
                                start=False,
                                stop=True,
                            )
                    # v-chain first (mask_rv gates the per-pair kv muls):
                    # m_v from psum on DVE, while ACT copies the k halves
                    vv = psA.tile([P, 8], F32, tag="vv", name="vv")
                    for t in range(2):
                        nc.vector.tensor_mul(
                            m_kv[:, t, HD:2 * HD], aps[(t, 1)][:],
                            wkvT_bf[:, t, HD:2 * HD],
                        )
                        nc.scalar.copy(ak_bf[:, t, :], aps[(t, 0)][:])
                    for g in range(4):
                        for tp in range(2):
                            nc.tensor.matmul(
                                vv[:, 4 + g:4 + g + 1],
                                m_kv[:, tp, HD + g * P:HD + (g + 1) * P],
                                ones_bf[:],
                                start=(tp == 0),
                                stop=(tp == 1),
                            )
                    nc.scalar.activation(
                        rv_col[:], vv[:, 4:8], AF.Sqrt, bias=eps_col[:, 0:1]
                    )
                    nc.vector.reciprocal(rv_col[:], rv_col[:])
                    for jp in range(4):
                        nc.vector.tensor_scalar_mul(
                            mask_rv[:, jp, :], mask_bf[:], rv_col[:, jp:jp + 1]
                        )
                    # k-chain: m_k from the bf16 copies (2x DVE rate)
                    for t in range(2):
                        nc.vector.tensor_mul(
                            m_kv[:, t, 0:HD], ak_bf[:, t, :], wkvT_bf[:, t, 0:HD]
                        )
                    for g in range(4):
                        for tp in range(2):
                            nc.tensor.matmul(
                                vv[:, g:g + 1],
                                m_kv[:, tp, g * P:(g + 1) * P],
                                ones_bf[:],
                                start=(tp == 0),
                                stop=(tp == 1),
                            )
                    nc.scalar.activation(
                        rk_col[:], vv[:, 0:4], AF.Sqrt, bias=eps_col[:, 0:1]
                    )
                    nc.vector.reciprocal(rk_col[:], rk_col[:])

                # per head-pair: kv^T block, mask*rv, bx*rk, W_eff accum
                with tc.tile_pool(name="psP", bufs=1, space="PSUM") as psP:
                    wps2 = [
                        psP.tile([P, OUT], F32, tag=f"weff{t}", name=f"wps{t}")
                        for t in range(2)
                    ]
                    for jp in range(4):
                        sl = slice(jp * P, (jp + 1) * P)
                        sd = psP.tile([P, P], F32, tag="sd", bufs=3, name="sd")
                        for tp in range(2):
                            nc.tensor.matmul(
                                sd[:],
                                wkvT_bf[:, tp, HD + jp * P:HD + (jp + 1) * P],
                                ak_bf[:, tp, sl],
                                start=(tp == 0),
                                stop=(tp == 1),
                            )
                        kv_bf = sm.tile([P, P], BF16, tag=f"kv{jp}", name=f"kv{jp}")
                        nc.vector.tensor_mul(kv_bf[:], sd[:], mask_rv[:, jp, :])
                        bx = psP.tile([P, OUT], F32, tag="bx", bufs=2, name="bx")
                        nc.tensor.matmul(
                            bx[:], kv_bf[:], woT_bf[:, jp, :], start=True, stop=True
                        )
                        bx_bf = sm.tile([P, OUT], BF16, tag=f"bxb{jp}", name=f"bxb{jp}")
                        if jp % 2 == 0:
                            nc.scalar.activation(
                                bx_bf[:], bx[:], AF.Copy,
                                scale=rk_col[:, jp:jp + 1],
                            )
                        else:
                            nc.vector.tensor_scalar_mul(
                                bx_bf[:], bx[:], rk_col[:, jp:jp + 1]
                            )
                        for t in range(2):
                            nc.tensor.matmul(
                                wps2[t][:],
                                wq_bf[:, jp, t * P:(t + 1) * P],
                                bx_bf[:],
                                start=(jp == 0),
                                stop=(jp == 3),
                            )
                    nc.scalar.copy(weff[:, 0, :], wps2[0][:])
                    nc.vector.tensor_copy(weff[:, 1, :], wps2[1][:])

            # ---- phase 3: out^T = W_eff^T u^T (bf16 PE stream) --------
            with (
                tc.tile_pool(name="opool", bufs=2) as opool,
                tc.tile_pool(name="pout", bufs=6, space="PSUM") as pout,
            ):
                for och in range(OCH):
                    osb = opool.tile([P, 2, CH_ROWS], BF16, tag="osb", name="osb")
                    for sg in range(NGROUPS // OCH):
                        s = och * (NGROUPS // OCH) + sg
                        for ob in range(2):
                            po = pout.tile([P, GROUP], F32, tag="po", name="po")
                            for t in range(2):
                                nc.tensor.matmul(
                                    po[:],
                                    weff[:, t, ob * P:(ob + 1) * P],
                                    uT[:, t, s * GROUP:(s + 1) * GROUP],
                                    start=(t == 0),
                                    stop=(t == 1),
                                )
                            dst = osb[:, ob, sg * GROUP:(sg + 1) * GROUP]
                            if ob == 0:
                                nc.vector.tensor_copy(dst, po[:])
                            else:
                                nc.scalar.copy(dst, po[:])
                        last = och == OCH - 1
                        if last or sg % 2 == 1:
                            # 2-group stripes; per-group on the last chunk
                            # so the final store tail is short
                            h0 = (sg if last else sg - 1) * GROUP
                            hn = GROUP if last else 2 * GROUP
                            nc.sync.dma_start(
                                out_d[:, och * 2 * CH_ROWS:(och + 1) * 2 * CH_ROWS]
                                .rearrange("p (a n) -> p a n", a=2)[
                                    :, :, h0:h0 + hn
                                ],
                                osb[:, :, h0:h0 + hn],
                            )

    nc.compile()
    return nc


_NC_CACHE = None


def _get_nc():
    global _NC_CACHE
    if _NC_CACHE is None:
        _NC_CACHE = build_nc()
    return _NC_CACHE


def make_in_maps(u_src, Wq, Wk, Wv, Wo):
    """Per-core input dicts. Core c = (batch c//2, half c%2); its own
    half of the grid axis is permuted to the front of u.  Everything is
    cast to bf16 and packed partition-major host-side."""
    bf = ml_dtypes.bfloat16
    wq_b = np.ascontiguousarray(
        Wq.reshape(4, P, C).transpose(1, 0, 2).reshape(P, 4 * C).astype(bf)
    )
    wkv = np.concatenate([Wk.T, Wv.T], axis=1)           # [C, 2HD]
    wkv_b = np.ascontiguousarray(
        wkv.reshape(2, P, 2 * HD).transpose(1, 0, 2).reshape(P, 4 * HD).astype(bf)
    )
    wot_b = np.ascontiguousarray(
        Wo.T.reshape(4, P, OUT).transpose(1, 0, 2).reshape(P, 4 * OUT).astype(bf)
    )
    in_maps = []
    for c in range(8):
        b, half = c // 2, c % 2
        ub = u_src[b]
        mine = ub[half * N_HALF:(half + 1) * N_HALF]
        other = ub[(1 - half) * N_HALF:(2 - half) * N_HALF]
        u_perm = np.concatenate([mine, other], axis=0)   # [N_FULL, C]
        u_r = np.empty((P, N_CHUNKS, SUBT, C1), dtype=bf)
        u_r[:, :, :, 0:C] = u_perm.reshape(N_CHUNKS, P, SUBT, C).transpose(
            1, 0, 2, 3
        ).astype(bf)
        u_r[:, :, :, C] = bf(1.0)
        u_r = u_r.reshape(P, U_ROW)
        in_maps.append({"u": u_r, "wq": wq_b, "wkv": wkv_b, "wot": wot_b})
    return in_maps


def assemble_output(results, bo):
    """Device emits out_r [P, OCH, 2, 2048] bf16 where element
    (p, och, a, j*128+pc) = out[row och*2048 + pc*16 + j, o=a*128+p]."""
    out = np.empty((4, N_FULL, OUT), dtype=np.float32)
    for c in range(8):
        b, half = c // 2, c % 2
        a = np.asarray(results[c]["out"]).astype(np.float32)
        a = a.reshape(P, OCH, 2, SUBT, P)        # [p, och, a, j, pc]
        a = a.transpose(1, 4, 3, 2, 0).reshape(N_HALF, OUT)
        out[b, half * N_HALF:(half + 1) * N_HALF] = a
    if np.any(bo):
        out += bo.reshape(1, 1, OUT)
    return out


def run(inputs, trace=False, tmpdir=None):
    """inputs: dict as from reference.setup_inputs(). Returns
    (full_output, BassKernelResults)."""
    u_src = np.asarray(inputs["u_src"], dtype=np.float32)
    Wq = np.asarray(inputs["Wq"], dtype=np.float32)
    Wk = np.asarray(inputs["Wk"], dtype=np.float32)
    Wv = np.asarray(inputs["Wv"], dtype=np.float32)
    Wo = np.asarray(inputs["Wo"], dtype=np.float32)
    bo = np.asarray(inputs["bo"], dtype=np.float32)
    nc = _get_nc()
    in_maps = make_in_maps(u_src, Wq, Wk, Wv, Wo)
    res = run_bass_kernel_spmd(
        nc, in_maps, core_ids=list(range(8)), trace=trace, tmpdir=tmpdir
    )
    return assemble_output(res.results, bo), res


def kernel(**inputs):
    out, _ = run(inputs, trace=False)
    return out
